# revision 3
# baseline (speedup 1.0000x reference)
"""KalmanNetNN single-step kernel on 8 TRN2 NeuronCores (Bass/Tile).

Sharding: tensor-parallel on the GRU hidden dim. Each core owns 900 of the
7200 hidden units: the matching r/z/n rows of Wih/Whh (row shard), the
matching columns of W2 (column shard). The small Kalman head (F, Hm,
normalize), W1 and the W3 tail run replicated on every core. One AllGather
of the 1152-element W2 partial sums is the only collective.

GEMVs run as fused multiply+reduce on the vector engine
(affine_mul_reduce) over natural row-major weight tiles [128, K] streamed
from HBM, with the activation vector DMA-broadcast across partitions.
The kg @ dy tail is algebraically folded: W3' = einsum('mnh,n->mh',
W3.reshape(24,12,H2), dy) is built on-device early (dy is available almost
immediately), so after l2 arrives the output is a single [24, H2] GEMV.
"""
import os
import sys

if "/opt/trn_rl_repo" not in sys.path:
    sys.path.insert(0, "/opt/trn_rl_repo")

import numpy as np

M, N = 24, 12
H1 = 2880
HID = 7200
H2 = 1152
NCORES = 8
U = HID // NCORES          # 900 hidden units per core
UT = 8                     # row tiles per gate (7 full + 1x4)
H1T = 23                   # l1 tiles (22 full + 1x64), padded to 23*128
H1P = H1T * 128            # 2944
W2T = H2 // 128            # 9

_CACHE = {}


def _build():
    import jax  # noqa: F401  (axon boot)
    import concourse.bacc as bacc
    import concourse.tile as tile
    import concourse.mybir as mybir

    f32 = mybir.dt.float32
    Act = mybir.ActivationFunctionType
    Alu = mybir.AluOpType

    nc = bacc.Bacc("TRN2", target_bir_lowering=False, debug=False,
                   num_devices=NCORES, enable_asserts=False)

    def din(name, shape):
        return nc.dram_tensor(name, shape, f32, kind="ExternalInput")

    whh_d = din("whh_s", [3 * U, HID])
    wih_d = din("wih_s", [3 * U, H1])
    w2_d = din("w2_s", [H2, U])
    w1_d = din("w1p", [H1P, 36])
    b1_d = din("b1_t", [128, H1T])
    bih_d = din("bih_t", [128, 3 * UT])
    bhh_d = din("bhh_t", [128, 3 * UT])
    hsh_d = din("hsh_t", [128, UT])
    b2_d = din("b2p", [H2])
    w3r_d = din("w3r", [N, M, H2])
    b3r_d = din("b3r", [M, N])
    f_d = din("F", [M, M])
    hm_d = din("Hm", [N, M])
    yt_d = din("yt", [N])
    mpost_d = din("m1x_post", [M])
    mold_d = din("m1x_old", [M])
    hn_d = din("hn", [HID])
    id_d = din("ident", [128, 128])
    out_d = nc.dram_tensor("out", [M], f32, kind="ExternalOutput")

    with tile.TileContext(nc) as tc:
        with (
            tc.tile_pool(name="kwhh", bufs=2) as kwhh,
            tc.tile_pool(name="kwih", bufs=2) as kwih,
            tc.tile_pool(name="w3blk", bufs=2) as w3blk,
            tc.tile_pool(name="big", bufs=1) as big,
            tc.tile_pool(name="sm", bufs=1) as sm,
            tc.tile_pool(name="ps", bufs=1, space="PSUM") as ps,
            tc.tile_pool(name="dram", bufs=1, space="DRAM") as dram,
        ):
            # ---- constants / broadcasts available up front ----
            ident = sm.tile([128, 128], f32)
            nc.sync.dma_start(out=ident, in_=id_d.ap())
            hn_b = big.tile([128, HID], f32)
            nc.sync.dma_start(out=hn_b, in_=hn_d.ap().partition_broadcast(128))

            # ---- Kalman head: m1x_prior, m1y, dy, dm1y, dm1x, kg_in ----
            f_sb = sm.tile([M, M], f32)
            nc.sync.dma_start(out=f_sb, in_=f_d.ap())
            mpost_b = sm.tile([M, M], f32)
            nc.sync.dma_start(out=mpost_b, in_=mpost_d.ap().partition_broadcast(M))
            mprior_p = sm.tile([M, 1], f32)
            nc.vector.affine_mul_reduce(out=f_sb, accum_out=mprior_p,
                                        in0=f_sb, in1=mpost_b, scale=1.0, bias=0.0)

            mprior_t = ps.tile([1, M], f32)
            nc.tensor.transpose(mprior_t, mprior_p, ident[0:M, 0:M])
            mprior_f = sm.tile([1, M], f32)
            nc.scalar.copy(mprior_f, mprior_t)

            hm_sb = sm.tile([N, M], f32)
            nc.sync.dma_start(out=hm_sb, in_=hm_d.ap())
            mprior_b = sm.tile([N, M], f32)
            nc.gpsimd.partition_broadcast(mprior_b, mprior_f[0:1, :])
            m1y_p = sm.tile([N, 1], f32)
            nc.vector.affine_mul_reduce(out=hm_sb, accum_out=m1y_p,
                                        in0=hm_sb, in1=mprior_b, scale=1.0, bias=0.0)
            m1y_t = ps.tile([1, N], f32)
            nc.tensor.transpose(m1y_t, m1y_p, ident[0:N, 0:N])
            m1y_f = sm.tile([1, N], f32)
            nc.scalar.copy(m1y_f, m1y_t)

            yt_f = sm.tile([1, N], f32)
            nc.sync.dma_start(out=yt_f, in_=yt_d.ap().unsqueeze(0))
            dy_f = sm.tile([1, N], f32)
            nc.vector.tensor_sub(dy_f, yt_f, m1y_f)

            kg_in = sm.tile([1, 36], f32)

            def normalize_into(dst, vec, n):
                scr = sm.tile([1, 32], f32, tag="normscr")
                ss = sm.tile([1, 1], f32, tag="normss")
                nc.scalar.activation(scr[:, 0:n], vec, Act.Square, accum_out=ss)
                nc.scalar.activation(ss, ss, Act.Sqrt)
                nc.vector.tensor_scalar_max(ss, ss, 1e-12)
                rcp = sm.tile([1, 1], f32, tag="normrcp")
                nc.vector.reciprocal(rcp, ss)
                nc.vector.tensor_scalar_mul(dst, vec, rcp)

            normalize_into(kg_in[0:1, 0:N], dy_f, N)

            mpost_f = sm.tile([1, M], f32)
            nc.sync.dma_start(out=mpost_f, in_=mpost_d.ap().unsqueeze(0))
            mold_f = sm.tile([1, M], f32)
            nc.sync.dma_start(out=mold_f, in_=mold_d.ap().unsqueeze(0))
            dx_f = sm.tile([1, M], f32)
            nc.vector.tensor_sub(dx_f, mpost_f, mold_f)
            normalize_into(kg_in[0:1, N:36], dx_f, M)

            kgin_b = sm.tile([128, 36], f32)
            nc.gpsimd.partition_broadcast(kgin_b, kg_in[0:1, :])
            dy_cols = sm.tile([M, N], f32)
            nc.gpsimd.partition_broadcast(dy_cols, dy_f[0:1, :])

            # ---- W3' = sum_n W3r[n] * dy[n]  (early; off critical path) ----
            w3p = sm.tile([M, H2], f32)
            for n in range(N):
                blk = w3blk.tile([M, H2], f32)
                nc.sync.dma_start(out=blk, in_=w3r_d.ap()[n])
                if n == 0:
                    nc.vector.tensor_scalar_mul(w3p, blk, dy_cols[:, 0:1])
                else:
                    nc.vector.scalar_tensor_tensor(
                        out=w3p, in0=blk, scalar=dy_cols[:, n:n + 1], in1=w3p,
                        op0=Alu.mult, op1=Alu.add)
            b3r_sb = sm.tile([M, N], f32)
            nc.sync.dma_start(out=b3r_sb, in_=b3r_d.ap())
            b3dy_t = sm.tile([M, N], f32)
            nc.vector.tensor_mul(b3dy_t, b3r_sb, dy_cols)
            b3dy = sm.tile([M, 1], f32)
            nc.vector.reduce_sum(b3dy, b3dy_t, axis=mybir.AxisListType.X)

            # ---- l1 = relu(W1 @ kg_in + b1) ----
            w1_sb = big.tile([128, H1T, 36], f32)
            nc.sync.dma_start(out=w1_sb,
                              in_=w1_d.ap().rearrange("(t p) c -> p t c", p=128))
            b1_sb = sm.tile([128, H1T], f32)
            nc.sync.dma_start(out=b1_sb, in_=b1_d.ap())
            l1p = sm.tile([128, H1T], f32)
            for t in range(H1T):
                nc.vector.affine_mul_reduce(
                    out=w1_sb[:, t, :], accum_out=l1p[:, t:t + 1],
                    in0=w1_sb[:, t, :], in1=kgin_b, scale=1.0, bias=0.0)
            nc.vector.tensor_add(l1p, l1p, b1_sb)
            nc.vector.tensor_scalar_max(l1p, l1p, 0.0)
            l1_t = ps.tile([H1T, 128], f32)
            nc.tensor.transpose(l1_t, l1p, ident)
            l1_ts = sm.tile([H1T, 128], f32)
            nc.scalar.copy(l1_ts, l1_t)
            l1_f = sm.tile([1, H1P], f32)
            nc.sync.dma_start(out=l1_f, in_=l1_ts)
            l1_b = big.tile([128, H1], f32)
            nc.gpsimd.partition_broadcast(l1_b, l1_f[0:1, 0:H1])

            # ---- big GEMV streams: gh = Whh_s @ hn, gi = Wih_s @ l1 ----
            gi = sm.tile([128, 3 * UT], f32)
            gh = sm.tile([128, 3 * UT], f32)
            nc.vector.memset(gi, 0.0)
            nc.vector.memset(gh, 0.0)

            for g in range(3):
                for t in range(UT):
                    rows = min(128, U - t * 128)
                    r0 = g * U + t * 128
                    b = g * UT + t
                    wt = kwhh.tile([128, HID], f32, tag="whh")
                    nc.sync.dma_start(out=wt[0:rows, :],
                                      in_=whh_d.ap()[r0:r0 + rows, :])
                    nc.vector.affine_mul_reduce(
                        out=wt[0:rows, :], accum_out=gh[0:rows, b:b + 1],
                        in0=wt[0:rows, :], in1=hn_b[0:rows, :],
                        scale=1.0, bias=0.0)
            for g in range(3):
                for t in range(UT):
                    rows = min(128, U - t * 128)
                    r0 = g * U + t * 128
                    b = g * UT + t
                    wt = kwih.tile([128, H1], f32, tag="wih")
                    nc.sync.dma_start(out=wt[0:rows, :],
                                      in_=wih_d.ap()[r0:r0 + rows, :])
                    nc.vector.affine_mul_reduce(
                        out=wt[0:rows, :], accum_out=gi[0:rows, b:b + 1],
                        in0=wt[0:rows, :], in1=l1_b[0:rows, :],
                        scale=1.0, bias=0.0)

            # ---- GRU gates on [128, UT] tiles ----
            bih_sb = sm.tile([128, 3 * UT], f32)
            nc.sync.dma_start(out=bih_sb, in_=bih_d.ap())
            bhh_sb = sm.tile([128, 3 * UT], f32)
            nc.sync.dma_start(out=bhh_sb, in_=bhh_d.ap())
            nc.vector.tensor_add(gi, gi, bih_sb)
            nc.vector.tensor_add(gh, gh, bhh_sb)
            S = UT
            rt = sm.tile([128, UT], f32)
            nc.vector.tensor_add(rt, gi[:, 0:S], gh[:, 0:S])
            nc.scalar.activation(rt, rt, Act.Sigmoid)
            zt = sm.tile([128, UT], f32)
            nc.vector.tensor_add(zt, gi[:, S:2 * S], gh[:, S:2 * S])
            nc.scalar.activation(zt, zt, Act.Sigmoid)
            nt = sm.tile([128, UT], f32)
            nc.vector.tensor_mul(nt, rt, gh[:, 2 * S:3 * S])
            nc.vector.tensor_add(nt, nt, gi[:, 2 * S:3 * S])
            nc.scalar.activation(nt, nt, Act.Tanh)
            hsh_sb = sm.tile([128, UT], f32)
            nc.sync.dma_start(out=hsh_sb, in_=hsh_d.ap())
            hnew = sm.tile([128, UT], f32)
            nc.vector.tensor_sub(hnew, hsh_sb, nt)
            nc.vector.tensor_mul(hnew, zt, hnew)
            nc.vector.tensor_add(hnew, hnew, nt)

            # flip h_new -> free layout, broadcast
            hnew_t = ps.tile([UT, 128], f32)
            nc.tensor.transpose(hnew_t, hnew, ident)
            hnew_ts = sm.tile([UT, 128], f32)
            nc.scalar.copy(hnew_ts, hnew_t)
            hnew_f = sm.tile([1, UT * 128], f32)
            nc.sync.dma_start(out=hnew_f, in_=hnew_ts)
            hnew_b = sm.tile([128, U], f32)
            nc.gpsimd.partition_broadcast(hnew_b, hnew_f[0:1, 0:U])

            # ---- W2 partials: p2 = W2_s @ h_new_s  -> AllGather -> sum ----
            w2_sb = big.tile([128, W2T, U], f32)
            nc.sync.dma_start(out=w2_sb,
                              in_=w2_d.ap().rearrange("(t p) c -> p t c", p=128))
            p2 = sm.tile([128, W2T], f32)
            for t in range(W2T):
                nc.vector.affine_mul_reduce(
                    out=w2_sb[:, t, :], accum_out=p2[:, t:t + 1],
                    in0=w2_sb[:, t, :], in1=hnew_b, scale=1.0, bias=0.0)

            bounce = dram.tile([H2], f32)
            agout = dram.tile([NCORES * H2], f32)
            nc.sync.dma_start(out=bounce.rearrange("(p t) -> p t", t=W2T), in_=p2)
            nc.gpsimd.collective_compute(
                "AllGather", Alu.bypass,
                replica_groups=[list(range(NCORES))],
                ins=[bounce.opt()], outs=[agout.opt()],
            )
            ag_sb = sm.tile([NCORES, H2], f32)
            nc.sync.dma_start(out=ag_sb,
                              in_=agout.rearrange("(r h) -> r h", h=H2))

            ones8 = sm.tile([NCORES, 1], f32)
            nc.vector.memset(ones8, 1.0)
            l2p = ps.tile([1, H2], f32)
            for s0, s1 in ((0, 512), (512, 1024), (1024, H2)):
                nc.tensor.matmul(l2p[:, s0:s1], ones8, ag_sb[:, s0:s1],
                                 start=True, stop=True)
            b2_sb = sm.tile([1, H2], f32)
            nc.sync.dma_start(out=b2_sb, in_=b2_d.ap().unsqueeze(0))
            l2_f = sm.tile([1, H2], f32)
            nc.vector.tensor_add(l2_f, l2p, b2_sb)
            nc.vector.tensor_scalar_max(l2_f, l2_f, 0.0)
            l2_b = sm.tile([M, H2], f32)
            nc.gpsimd.partition_broadcast(l2_b, l2_f[0:1, :])

            # ---- out = m1x_prior + W3' @ l2 + b3r @ dy ----
            facc = sm.tile([M, 1], f32)
            nc.vector.affine_mul_reduce(out=w3p, accum_out=facc,
                                        in0=w3p, in1=l2_b, scale=1.0, bias=0.0)
            nc.vector.tensor_add(facc, facc, b3dy)
            nc.vector.tensor_add(facc, facc, mprior_p)
            nc.sync.dma_start(out=out_d.ap().unsqueeze(1), in_=facc)

    nc.compile()
    return nc


def _prep_inputs(inp):
    """Shard + lay out the full inputs for the 8 cores (host-side numpy)."""
    f32 = np.float32
    Whh = np.asarray(inp["Whh"], f32)
    Wih = np.asarray(inp["Wih"], f32)
    W1 = np.asarray(inp["W1"], f32)
    W2 = np.asarray(inp["W2"], f32)
    W3 = np.asarray(inp["W3"], f32)
    b1 = np.asarray(inp["b1"], f32)
    b2 = np.asarray(inp["b2"], f32)
    b3 = np.asarray(inp["b3"], f32)
    bih = np.asarray(inp["bih"], f32)
    bhh = np.asarray(inp["bhh"], f32)
    hn = np.asarray(inp["hn"], f32)

    # W1 padded to H1P rows, tiled bias
    w1p = np.zeros((H1P, 36), f32)
    w1p[:H1, :] = W1
    b1pad = np.zeros(H1P, f32)
    b1pad[:H1] = b1
    b1_t = np.ascontiguousarray(b1pad.reshape(H1T, 128).T)

    # permutation for l2 ordering: bounce[j] with j = p*W2T + t holds W2 row
    # r = t*128 + p
    j = np.arange(H2)
    p_, t_ = j // W2T, j % W2T
    r_of_j = t_ * 128 + p_
    b2p = np.ascontiguousarray(b2[r_of_j])
    w3perm = W3[:, r_of_j]                       # [288, H2]
    w3r = np.ascontiguousarray(
        w3perm.reshape(M, N, H2).transpose(1, 0, 2))  # [N, M, H2]
    b3r = np.ascontiguousarray(b3.reshape(M, N))

    common = {
        "w1p": w1p, "b1_t": b1_t, "b2p": b2p, "w3r": w3r, "b3r": b3r,
        "F": np.asarray(inp["F"], f32),
        "Hm": np.asarray(inp["Hm"], f32),
        "yt": np.asarray(inp["yt"], f32),
        "m1x_post": np.asarray(inp["m1x_posterior"], f32).reshape(M),
        "m1x_old": np.asarray(inp["m1x_prior_old"], f32).reshape(M),
        "hn": hn,
        "ident": np.eye(128, dtype=f32),
    }

    def tile_units(vec_u):  # [U] -> [128, UT] padded
        out = np.zeros((128, UT), f32)
        padded = np.zeros(UT * 128, f32)
        padded[:U] = vec_u
        out[:] = padded.reshape(UT, 128).T
        return np.ascontiguousarray(out)

    def tile_gates(vec_3hid, c):  # per-core [128, 3*UT]
        cols = []
        for g in range(3):
            seg = vec_3hid[g * HID + c * U: g * HID + (c + 1) * U]
            cols.append(tile_units(seg))
        return np.ascontiguousarray(np.concatenate(cols, axis=1))

    in_maps = []
    for c in range(NCORES):
        sl = slice(c * U, (c + 1) * U)
        whh_s = np.ascontiguousarray(
            np.concatenate([Whh[g * HID + c * U: g * HID + (c + 1) * U, :]
                            for g in range(3)], axis=0))
        wih_s = np.ascontiguousarray(
            np.concatenate([Wih[g * HID + c * U: g * HID + (c + 1) * U, :]
                            for g in range(3)], axis=0))
        m = dict(common)
        m.update({
            "whh_s": whh_s,
            "wih_s": wih_s,
            "w2_s": np.ascontiguousarray(W2[:, sl]),
            "bih_t": tile_gates(bih, c),
            "bhh_t": tile_gates(bhh, c),
            "hsh_t": tile_units(hn[sl]),
        })
        in_maps.append(m)
    return in_maps


def kernel(**inputs) -> np.ndarray:
    import concourse.bass_utils as bass_utils

    if "nc" not in _CACHE:
        _CACHE["nc"] = _build()
    nc = _CACHE["nc"]
    in_maps = _prep_inputs(inputs)

    trace = os.environ.get("BASS_KERNEL_TRACE", "0") == "1"
    if trace:
        bass_utils.upload_artifacts = lambda tmpdir: tmpdir
    res = bass_utils.run_bass_kernel_spmd(
        nc, in_maps, core_ids=list(range(NCORES)), trace=trace,
        tmpdir=os.environ.get("BASS_KERNEL_TRACE_DIR"),
    )
    if trace:
        _CACHE["exec_time_ns"] = res.exec_time_ns
        print(f"HW exec time: {res.exec_time_ns} ns")
    return np.asarray(res.results[0]["out"], np.float32)
